# revision 15
# baseline (speedup 1.0000x reference)
"""Trainium2 Bass kernel for a 12-qubit batched PennyLane-style circuit.

Circuit (per batch sample), 4 layers:
  - data-encoding RY,RX,RZ,RY per wire (per-sample angles) followed by a
    fixed Rot per wire  -> folded on host into ONE 2x2 SU(2) gate G[l,q,b]
  - CRot entangling ring CRot(q, q+1 mod 12), fixed per layer.
Then <Z_i> for each of the 12 wires.

Schedule optimization: within each layer l>=1, the per-wire gate G[l,t]
(t=1..11) is delayed and merged into the following CRot(t-1, t) as a
"uniformly-controlled pair": ctrl=0 half applies G, ctrl=1 half applies
U@G.  This rewrites the full state once instead of (full + half) twice,
cutting total gate MACs ~27%.  Layer-1's 12 gates acting on |0..0> become
a direct Kronecker build of the product state.

Distribution: pure data parallel over the batch. 4096 samples -> 8 cores
x 512 samples; each core holds its 512x4096 complex statevector in SBUF
as fp32 re/im planes, batch on partitions (4 tiles of 128 samples).

Engine split per site application (knob-tuned): TensorE applies gates as
diag-matmul accumulation into PSUM (4 matmuls of 512 per output chunk)
with ScalarE wide evictions; VectorE applies them as
activation-start + scalar_tensor_tensor chains writing a ping-pong
destination buffer directly (no copy-backs).  Plain CRots run in-place
on TensorE (eviction is the write-back; untouched ctrl=0 half stays).
"""

import numpy as np

import concourse.bass as bass
import concourse.bacc as bacc
import concourse.mybir as mybir
from concourse.tile import TileContext
from concourse.bass_utils import run_bass_kernel_spmd

F32 = mybir.dt.float32
F32R = mybir.dt.float32r
F16 = mybir.dt.float16
ALU = mybir.AluOpType

N_QUBITS = 12
DIM = 4096            # 2**12
B_FULL = 4096
N_CORES = 8
B_CORE = B_FULL // N_CORES   # 512
NBT = B_CORE // 128          # 4 batch tiles of 128 samples

# coefficient plane order per gate (12 per-partition scalars)
#  a=[0,0] b=[0,1] c=[1,0] d=[1,1] of the 2x2 complex gate
CO_ARE, CO_AIM, CO_MAIM, CO_BRE, CO_BIM, CO_MBIM, \
    CO_CRE, CO_CIM, CO_MCIM, CO_DRE, CO_DIM, CO_MDIM = range(12)
NCO = 12

NG = 81               # per-sample gates: 12 kron + 3 layers * (1 + 22)
GCO_W = NG * NCO * NBT
NCC = 15              # const crot gates: 12 layer-0 + 3 wrap
CCO_W = NCC * NCO

# ---------------------------------------------------------------------------
# engine plan knobs
PLAN = ("pe", "pe", "dve")         # per-sample site applications
CROT_PLAN = ("pe",)                # plain crot sites
COPY_ROT = ("act", "dve")          # identity copies for native plain crots
DIAG_ROT = ("dve", "act")          # diag builds
EVICT_ROT = ("act",)               # psum evictions
SQ_ROT = ("act",)                  # observable squares
KRON_PROD_ROT = ("act", "act", "dve")  # kron product terms
OBS_GPS_BT = (1, 3)                # batch tiles whose obs folds run on GpSimd

# TERMS[plane] = ordered (ci, src_idx); src order (s0re, s0im, s1re, s1im);
# planes ordered (y0re, y0im, y1re, y1im)
TERMS = (
    ((CO_ARE, 0), (CO_MAIM, 1), (CO_BRE, 2), (CO_MBIM, 3)),
    ((CO_ARE, 1), (CO_AIM, 0), (CO_BRE, 3), (CO_BIM, 2)),
    ((CO_CRE, 0), (CO_MCIM, 1), (CO_DRE, 2), (CO_MDIM, 3)),
    ((CO_CRE, 1), (CO_CIM, 0), (CO_DRE, 3), (CO_DIM, 2)),
)
# emission order grouping matmuls by diag (amortize LDWEIGHTS) with all s0
# reads in the first half (lets in-place evictions of the s0-destined planes
# start mid-pass): (ci, plane, src)
PE_ORDER = (
    (CO_ARE, 0, 0), (CO_ARE, 1, 1), (CO_MAIM, 0, 1), (CO_AIM, 1, 0),
    (CO_CRE, 2, 0), (CO_CRE, 3, 1), (CO_MCIM, 2, 1), (CO_CIM, 3, 0),
    (CO_BRE, 0, 2), (CO_BRE, 1, 3), (CO_MBIM, 0, 3), (CO_BIM, 1, 2),
    (CO_DRE, 2, 2), (CO_DRE, 3, 3), (CO_MDIM, 2, 3), (CO_DIM, 3, 2),
)


def _sched():
    """Gate schedule shared by host coeff packing and device emission.
    ('crot', c, t, cidx) | ('full', q, g) | ('pair', c, t, g0, g1).
    Each layer's G[l,t] (t>=1) merges into crot(t-1,t) of layer l; the wrap
    sites (stride-2 access) stay plain crots and G[l,0] stays a full site."""
    ops = []
    cc = 0
    for c in range(12):
        ops.append(("crot", c, (c + 1) % 12, cc))
        cc += 1
    g = 12
    for _l in (1, 2, 3):
        ops.append(("full", 0, g))
        g += 1
        for c in range(11):
            ops.append(("pair", c, c + 1, g, g + 1))
            g += 2
        ops.append(("crot", 11, 0, cc))
        cc += 1
    assert g == NG and cc == NCC
    return tuple(ops)


OPS_SCHED = _sched()

# ---------------------------------------------------------------------------
# host-side gate algebra (numpy, trivially cheap vs the device work)
# ---------------------------------------------------------------------------


def _rz(t):
    e = np.exp(-0.5j * t)
    z = np.zeros_like(e)
    return np.stack([np.stack([e, z], -1), np.stack([z, np.conj(e)], -1)], -2)


def _ry(t):
    c = np.cos(t / 2).astype(np.complex128)
    s = np.sin(t / 2).astype(np.complex128)
    return np.stack([np.stack([c, -s], -1), np.stack([s, c], -1)], -2)


def _rx(t):
    c = np.cos(t / 2).astype(np.complex128)
    s = np.sin(t / 2).astype(np.complex128)
    return np.stack([np.stack([c, -1j * s], -1), np.stack([-1j * s, c], -1)], -2)


def _rot(phi, theta, omega):
    # PennyLane Rot = RZ(omega) @ RY(theta) @ RZ(phi)
    return _rz(omega) @ _ry(theta) @ _rz(phi)


def _coef_planes(g):
    """g: [..., 2, 2] complex -> [..., 12] float32 coefficient planes."""
    a, b = g[..., 0, 0], g[..., 0, 1]
    c, d = g[..., 1, 0], g[..., 1, 1]
    cols = [a.real, a.imag, -a.imag, b.real, b.imag, -b.imag,
            c.real, c.imag, -c.imag, d.real, d.imag, -d.imag]
    return np.stack(cols, -1).astype(np.float32)


def _host_coeffs(x, q_params_rot, q_params_enta):
    """Returns (gco [NG,12,B] f32 per-sample planes, cco [NCC,12] f32)."""
    x = np.asarray(x, np.float64)
    pr = np.asarray(q_params_rot, np.float64)
    pe = np.asarray(q_params_enta, np.float64)
    B = x.shape[0]

    # per-sample encoding gate per wire: RY(x3) RZ(x2) RX(x1) RY(x0)
    enc = np.einsum('qbij,qbjk->qbik',
                    _ry(x[:, 3, :].T),
                    np.einsum('qbij,qbjk->qbik', _rz(x[:, 2, :].T),
                              np.einsum('qbij,qbjk->qbik',
                                        _rx(x[:, 1, :].T), _ry(x[:, 0, :].T))))
    rot = _rot(pr[..., 0], pr[..., 1], pr[..., 2])      # [L,Q,2,2]
    G = np.einsum('lqij,qbjk->lqbik', rot, enc)         # [L,Q,B,2,2]
    U = _rot(pe[..., 0], pe[..., 1], pe[..., 2])        # [L,Q,2,2]

    gates = np.empty((NG, B, 2, 2), np.complex128)
    gates[0:12] = G[0]
    g = 12
    for l in (1, 2, 3):
        gates[g] = G[l, 0]
        g += 1
        for c in range(11):
            t = c + 1
            gates[g] = G[l, t]                       # P0 (ctrl=0 half)
            gates[g + 1] = np.einsum('ij,bjk->bik', U[l, c], G[l, t])  # P1
            g += 2
    cgates = np.empty((NCC, 2, 2), np.complex128)
    cgates[0:12] = U[0]
    for i, l in enumerate((1, 2, 3)):
        cgates[12 + i] = U[l, 11]

    gco = np.moveaxis(_coef_planes(gates), -1, 1)       # [NG,12,B]
    cco = _coef_planes(cgates)                          # [NCC,12]
    return gco.astype(np.float32), cco.astype(np.float32)


# ---------------------------------------------------------------------------
# bass program
# ---------------------------------------------------------------------------


class _Prog:
    def __init__(self):
        nc = bacc.Bacc("TRN2", target_bir_lowering=False, debug=False)
        self.nc = nc
        self.gco_d = nc.declare_dram_parameter("gcoef", [128, GCO_W], F32,
                                               isOutput=False)
        self.cco_d = nc.declare_dram_parameter("ccoef", [128, CCO_W], F32,
                                               isOutput=False)
        self.idn_d = nc.declare_dram_parameter("ident", [128, 128], F16,
                                               isOutput=False)
        self.z_d = nc.declare_dram_parameter("z", [B_CORE, N_QUBITS], F32,
                                             isOutput=True)
        self._pctr = 0
        self._crotctr = 0
        self._cpctr = 0
        self._dctr = 0
        self._ectr = 0
        self._sqctr = 0
        self._kpctr = 0
        with TileContext(nc) as tc:
            self.tc = tc
            with tc.tile_pool(name="main", bufs=1) as pool, \
                    tc.tile_pool(name="dpool", bufs=96) as dpool, \
                    tc.tile_pool(name="psum", bufs=4, space="PSUM") as ppool:
                self.dpool = dpool
                self.ppool = ppool
                # state: bt-major, then comp (0=re 1=im), then 4096 amplitudes
                self.ST = pool.tile([128, NBT * 2 * DIM], F16, tag="state")
                self.SP = [pool.tile([128, 2 * DIM], F16, name=f"sp{i}",
                                     tag=f"sp{i}") for i in range(NBT)]
                self.GC = pool.tile([128, GCO_W], F32, tag="gc")
                self.CC = pool.tile([128, CCO_W], F32, tag="cc")
                self.I128 = pool.tile([128, 128], F16, tag="ident")
                self.ZT = [pool.tile([128, 16], F32, name=f"z{bt}",
                                     tag=f"z{bt}") for bt in range(NBT)]
                self.OT = pool.tile([128, NBT * 2048], F32, tag="obst")
                self.cur = [(self.ST, bt * 2 * DIM) for bt in range(NBT)]
                self.spare = [(self.SP[i], 0) for i in range(NBT)]

                nc.sync.dma_start(out=self.GC[:], in_=self.gco_d[:])
                nc.sync.dma_start(out=self.CC[:], in_=self.cco_d[:])
                nc.sync.dma_start(out=self.I128[:], in_=self.idn_d[:])

                self._emit_circuit()

                for bt in range(NBT):
                    nc.sync.dma_start(
                        out=self.z_d[bt * 128:(bt + 1) * 128, :],
                        in_=self.ZT[bt][:, 0:N_QUBITS])
        nc.compile()

    # ---- AP helpers -----------------------------------------------------

    def plane(self, buf, comp):
        """[128, 4096] AP of one re/im plane of a (tile, offset) buffer."""
        t, off = buf
        o = off + comp * DIM
        return t[:, o:o + DIM]

    def half_slices(self, buf, q):
        """(s0re, s0im, s1re, s1im) pair slices for a 1q gate on wire q."""
        s = 1 << (11 - q)
        out = []
        for bit in (0, 1):
            for comp in (0, 1):
                p = self.plane(buf, comp).rearrange(
                    "p (a c r) -> p a c r", c=2, r=s)
                out.append(p[:, :, bit, :])
        return (out[0], out[1], out[2], out[3])

    def pair_slices(self, buf, c, t, cbit):
        """(s0re, s0im, s1re, s1im): ctrl bit c == cbit, pair over target t."""
        out = []
        if c < t:      # adjacent, c = t-1
            st = 1 << (11 - t)
            for tbit in (0, 1):
                for comp in (0, 1):
                    p = self.plane(buf, comp).rearrange(
                        "p (a cc tt r) -> p a cc tt r", cc=2, tt=2, r=st)
                    out.append(p[:, :, cbit, tbit, :])
        else:          # wrap: c=11 (LSB), t=0 (MSB)
            for tbit in (0, 1):
                for comp in (0, 1):
                    p = self.plane(buf, comp).rearrange(
                        "p (tt a cc) -> p tt a cc", tt=2, cc=2)
                    out.append(p[:, tbit, :, cbit])
        return (out[0], out[1], out[2], out[3])

    def gco(self, bt, g, ci):
        idx = (g * NCO + ci) * NBT + bt
        return self.GC[:, idx:idx + 1]

    def cco(self, cc, ci):
        idx = cc * NCO + ci
        return self.CC[:, idx:idx + 1]

    @staticmethod
    def _chunk(view, idx, csz):
        """csz-wide column chunk of a slice-AP shaped [128, w] or [128,n,s]."""
        shp = view.shape[1:]
        if len(shp) == 1:
            return view[:, idx * csz:(idx + 1) * csz]
        n, s = shp
        if s >= csz:
            m = s // csz
            return view[:, idx // m, (idx % m) * csz:(idx % m + 1) * csz]
        na = csz // s
        return view[:, idx * na:(idx + 1) * na, :]

    @staticmethod
    def _match(ps_ap, dst):
        """View of a flat [128, w] psum AP matching dst's chunk geometry."""
        shp = dst.shape[1:]
        if len(shp) == 1:
            return ps_ap
        return ps_ap.rearrange("p (a r) -> p a r", r=shp[1])

    # ---- gate emission --------------------------------------------------

    def _build_diags(self, co):
        nc = self.nc
        AF = mybir.ActivationFunctionType
        D = {}
        for ci in range(NCO):
            d = self.dpool.tile([128, 128], F16, name="dg", tag="dg")
            eng = DIAG_ROT[self._dctr % len(DIAG_ROT)]
            self._dctr += 1
            if eng == "act":
                nc.scalar.activation(d[:], self.I128[:], AF.Copy, scale=co(ci))
            else:
                nc.vector.tensor_scalar(d[:], self.I128[:], co(ci),
                                        None, ALU.mult)
            D[ci] = d
        return D

    def _evict(self, ps_tile, dst):
        nc = self.nc
        ev = EVICT_ROT[self._ectr % len(EVICT_ROT)]
        self._ectr += 1
        src = self._match(ps_tile[:], dst)
        if ev == "act":
            nc.scalar.copy(dst, src)
        elif ev == "gps":
            nc.gpsimd.tensor_copy(out=dst, in_=src)
        else:
            nc.vector.tensor_copy(out=dst, in_=src)

    def _apply(self, s, d, co, width, plan, D=None, inplace=False):
        """Apply one 2x2 complex gate: reads slices s, writes slices d."""
        nc = self.nc
        AF = mybir.ActivationFunctionType
        if plan == "pe":
            if D is None:
                D = self._build_diags(co)
            ncp = width // 1024
            for cp in range(ncp):
                if ncp > 1:
                    ss = [self._chunk(v, cp, 1024) for v in s]
                    dd = [self._chunk(v, cp, 1024) for v in d]
                else:
                    ss, dd = s, d
                ps = [self.ppool.tile([128, 1024], F32, name="pp", tag="pp")
                      for _ in range(4)]
                if inplace:
                    # s0-reads first so s0-destined evictions can overlap the
                    # tail matmuls; evict only at end (dst aliases src)
                    kc = {}
                    for (ci, pl, si) in PE_ORDER:
                        for ch in range(2):
                            k = kc.get((pl, ch), 0)
                            nc.tensor.matmul(
                                out=ps[pl][:, ch * 512:(ch + 1) * 512],
                                lhsT=D[ci][:],
                                rhs=self._chunk(ss[si], ch, 512),
                                start=(k == 0), stop=(k == 3))
                            kc[(pl, ch)] = k + 1
                    for pl in range(4):
                        self._evict(ps[pl], dd[pl])
                else:
                    # plane-major; evict each plane as soon as it completes
                    # so PSUM slots free early and the pipe stays full
                    for pl in range(4):
                        for k, (ci, si) in enumerate(TERMS[pl]):
                            for ch in range(2):
                                nc.tensor.matmul(
                                    out=ps[pl][:, ch * 512:(ch + 1) * 512],
                                    lhsT=D[ci][:],
                                    rhs=self._chunk(ss[si], ch, 512),
                                    start=(k == 0), stop=(k == 3))
                        self._evict(ps[pl], dd[pl])
        else:
            # starts first (Sc), then stt rounds interleaved across planes so
            # the DVE queue never head-of-line blocks on one serial chain
            for pl in range(4):
                ci0, si0 = TERMS[pl][0]
                nc.scalar.activation(d[pl], s[si0], AF.Copy, scale=co(ci0))
            for k in (1, 2, 3):
                for pl in range(4):
                    ci, si = TERMS[pl][k]
                    nc.vector.scalar_tensor_tensor(
                        d[pl], s[si], co(ci), d[pl], ALU.mult, ALU.add)

    def _crot_native(self, bt, c, t, co):
        """Native plain crot: identity-copy ctrl=0 half, chains on ctrl=1."""
        nc = self.nc
        src, dst = self.cur[bt], self.spare[bt]
        s0 = self.pair_slices(src, c, t, 0)
        d0 = self.pair_slices(dst, c, t, 0)
        for k in range(4):
            eng = COPY_ROT[self._cpctr % len(COPY_ROT)]
            self._cpctr += 1
            if eng == "act":
                nc.scalar.copy(d0[k], s0[k])
            elif eng == "gps":
                nc.gpsimd.tensor_copy(out=d0[k], in_=s0[k])
            else:
                nc.vector.tensor_copy(out=d0[k], in_=s0[k])
        s1 = self.pair_slices(src, c, t, 1)
        d1 = self.pair_slices(dst, c, t, 1)
        self._apply(s1, d1, co, DIM // 4, "dve")
        self.cur[bt], self.spare[bt] = self.spare[bt], self.cur[bt]

    def _kron_init_all(self):
        """Build the layer-0 post-1q product states directly:
        state = kron_q (G[0,q] @ e0).  Step-major across batch tiles so the
        four serial doubling chains overlap."""
        nc = self.nc
        AF = mybir.ActivationFunctionType
        eng = nc.vector

        def prod(dst, src, sc):
            pe = KRON_PROD_ROT[self._kpctr % len(KRON_PROD_ROT)]
            self._kpctr += 1
            if pe == "act":
                nc.scalar.activation(dst, src, AF.Copy, scale=sc)
            else:
                eng.tensor_scalar(dst, src, sc, None, ALU.mult)

        for bt in range(NBT):
            re = self.plane(self.cur[bt], 0)
            im = self.plane(self.cur[bt], 1)
            co = lambda q, ci: self.gco(bt, q, ci)
            sp_t, sp_off = self.spare[bt]
            base = sp_off
            t0 = sp_t[:, base:base + 512]
            t1 = sp_t[:, base + 512:base + 1024]
            eng.tensor_copy(out=re[:, 0:1], in_=co(11, CO_ARE))
            eng.tensor_copy(out=im[:, 0:1], in_=co(11, CO_AIM))
            eng.tensor_copy(out=re[:, 1:2], in_=co(11, CO_CRE))
            eng.tensor_copy(out=im[:, 1:2], in_=co(11, CO_CIM))
            w = 2
            for q in range(10, -1, -1):
                csz = min(w, 512)
                for k in range(w // csz):
                    sl = slice(k * csz, (k + 1) * csz)
                    su = slice(w + k * csz, w + (k + 1) * csz)
                    ore, oim = re[:, sl], im[:, sl]
                    tt0, tt1 = t0[:, 0:csz], t1[:, 0:csz]
                    # upper half <- (c) * old (written before old clobbered)
                    prod(tt0, ore, co(q, CO_CRE))
                    eng.scalar_tensor_tensor(re[:, su], oim, co(q, CO_MCIM),
                                             tt0, ALU.mult, ALU.add)
                    prod(tt1, ore, co(q, CO_CIM))
                    eng.scalar_tensor_tensor(im[:, su], oim, co(q, CO_CRE),
                                             tt1, ALU.mult, ALU.add)
                    # lower half <- (a) * old, in place
                    prod(tt0, ore, co(q, CO_ARE))
                    prod(tt1, ore, co(q, CO_AIM))
                    eng.scalar_tensor_tensor(ore, oim, co(q, CO_MAIM),
                                             tt0, ALU.mult, ALU.add)
                    eng.scalar_tensor_tensor(oim, oim, co(q, CO_ARE),
                                             tt1, ALU.mult, ALU.add)
                w *= 2

    def _obs_ctx(self, bt):
        re = self.plane(self.cur[bt], 0)
        im = self.plane(self.cur[bt], 1)
        base = bt * 2048
        return re, im, self.OT[:, base:base + 1024], self.OT[:, base + 1024:base + 2048]

    def _observables(self, bt):
        """probs = re^2+im^2 (overwrites re), then the 12 <Z_q> per wire."""
        nc = self.nc
        eng = nc.vector
        e = nc.gpsimd if bt in OBS_GPS_BT else eng
        AF = mybir.ActivationFunctionType
        re, im, t0, t1 = self._obs_ctx(bt)
        for h in range(4):
            sl = slice(h * 1024, (h + 1) * 1024)
            nc.scalar.activation(t0, re[:, sl], AF.Square)
            nc.scalar.activation(t1, im[:, sl], AF.Square)
            e.tensor_tensor(re[:, sl], t0, t1, ALU.add)
        # fold out qubits MSB-first; z_q = sum(lo half) - sum(hi half)
        w = DIM
        for q in range(N_QUBITS):
            h = w // 2
            lo, hi = re[:, 0:h], re[:, h:w]
            if h > 1024:  # only q=0: do the diff/reduce in two chunks
                for k in range(2):
                    sk = slice(k * 1024, (k + 1) * 1024)
                    e.tensor_tensor(t0, lo[:, sk], hi[:, sk], ALU.subtract)
                    eng.tensor_reduce(out=self.ZT[bt][:, 12 + k:13 + k],
                                      in_=t0, op=ALU.add,
                                      axis=mybir.AxisListType.X)
                eng.tensor_tensor(self.ZT[bt][:, q:q + 1],
                                  self.ZT[bt][:, 12:13],
                                  self.ZT[bt][:, 13:14], ALU.add)
            else:
                e.tensor_tensor(t0[:, 0:h], lo, hi, ALU.subtract)
                eng.tensor_reduce(out=self.ZT[bt][:, q:q + 1],
                                  in_=t0[:, 0:h], op=ALU.add,
                                  axis=mybir.AxisListType.X)
            if q < N_QUBITS - 1:
                for k in range(max(1, h // 1024)):
                    sk = slice(k * 1024, min((k + 1) * 1024, h))
                    e.tensor_tensor(lo[:, sk], lo[:, sk], hi[:, sk], ALU.add)
            w = h

    def _emit_circuit(self):
        # pre-warm: diag builds are state-independent; emit the first pe-crot
        # sites' builds before kron so the PE can start the moment kron(bt0)
        # lands
        crot_h = {}
        ci = 0
        for op in OPS_SCHED:
            if op[0] != "crot":
                continue
            plan = CROT_PLAN[ci % len(CROT_PLAN)]
            if plan == "pe" and len(crot_h) < 2:
                cc = op[3]
                co = lambda x, cc=cc: self.cco(cc, x)
                crot_h[op] = {"D": self._build_diags(co)}
            ci += 1
        self._kron_init_all()
        # app pipeline: emit each app's diag builds (state-independent) one
        # app ahead of its gate ops, hiding build latency behind prior work
        prevq = []

        def push(build_fn, apply_fn):
            D = build_fn() if build_fn else None
            if len(prevq) >= 2:
                prevq.pop(0)()
            prevq.append(lambda f=apply_fn, D=D: f(D))

        for oi, op in enumerate(OPS_SCHED):
            tail = oi >= len(OPS_SCHED) - 1
            if op[0] == "crot":
                _, c, t, cc = op
                plan = CROT_PLAN[self._crotctr % len(CROT_PLAN)]
                self._crotctr += 1
                if tail:
                    plan = "pe"
                co = lambda ci, cc=cc: self.cco(cc, ci)
                if plan == "pe":
                    h = crot_h.get(op, {})
                    for bt in range(NBT):
                        bf = None
                        if bt == 0:
                            bf = (lambda co=co, h=h:
                                  h.setdefault("D", self._build_diags(co)))

                        def ap(D, bt=bt, c=c, t=t, co=co, h=h):
                            s = self.pair_slices(self.cur[bt], c, t, 1)
                            self._apply(s, s, co, DIM // 4, "pe",
                                        D=h["D"], inplace=True)
                        push(bf, ap)
                        if tail:
                            push(None, lambda D, bt=bt: self._observables(bt))
                else:
                    for bt in range(NBT):
                        push(None, lambda D, bt=bt, c=c, t=t, co=co:
                             self._crot_native(bt, c, t, co))
                        if tail:
                            push(None, lambda D, bt=bt: self._observables(bt))
            elif op[0] == "full":
                _, q, g = op
                for bt in range(NBT):
                    plan = PLAN[self._pctr % len(PLAN)]
                    self._pctr += 1
                    co = lambda ci, bt=bt, g=g: self.gco(bt, g, ci)
                    bf = ((lambda co=co: self._build_diags(co))
                          if plan == "pe" else None)

                    def ap(D, bt=bt, q=q, co=co, plan=plan):
                        src, dst = self.cur[bt], self.spare[bt]
                        s = self.half_slices(src, q)
                        d = self.half_slices(dst, q)
                        self._apply(s, d, co, DIM // 2, plan, D=D)
                        self.cur[bt], self.spare[bt] = self.spare[bt], self.cur[bt]
                    push(bf, ap)
            else:
                _, c, t, g0, g1 = op
                for bt in range(NBT):
                    plan = PLAN[self._pctr % len(PLAN)]
                    self._pctr += 1
                    if tail:
                        plan = "pe"
                    co0 = lambda ci, bt=bt, g=g0: self.gco(bt, g, ci)
                    co1 = lambda ci, bt=bt, g=g1: self.gco(bt, g, ci)
                    bf = ((lambda co0=co0, co1=co1:
                           (self._build_diags(co0), self._build_diags(co1)))
                          if plan == "pe" else None)

                    def ap(D, bt=bt, c=c, t=t, co0=co0, co1=co1, plan=plan):
                        src, dst = self.cur[bt], self.spare[bt]
                        D0, D1 = D if D is not None else (None, None)
                        for cbit, co_, DD in ((0, co0, D0), (1, co1, D1)):
                            s = self.pair_slices(src, c, t, cbit)
                            d = self.pair_slices(dst, c, t, cbit)
                            self._apply(s, d, co_, DIM // 4, plan, D=DD)
                        self.cur[bt], self.spare[bt] = self.spare[bt], self.cur[bt]
                    push(bf, ap)
        for f in prevq:
            f()


_PROG_CACHE = None


def _get_prog():
    global _PROG_CACHE
    if _PROG_CACHE is None:
        _PROG_CACHE = _Prog()
    return _PROG_CACHE


def _run(inputs, trace=False):
    x = np.asarray(inputs["x"], np.float32)
    gco, cco = _host_coeffs(x, inputs["q_params_rot"], inputs["q_params_enta"])
    in_maps = []
    cco_tile = np.broadcast_to(
        cco.reshape(1, CCO_W), (128, CCO_W)).copy()
    for core in range(N_CORES):
        lo = core * B_CORE
        g = gco[:, :, lo:lo + B_CORE]                    # [NG,12,512]
        g = g.reshape(NG, NCO, NBT, 128)                 # [NG,12,bt,p]
        g = np.ascontiguousarray(np.moveaxis(g, -1, 0))  # [p,NG,12,bt]
        in_maps.append({
            "gcoef": g.reshape(128, GCO_W),
            "ccoef": cco_tile,
            "ident": np.eye(128, dtype=np.float16),
        })
    prog = _get_prog()
    res = run_bass_kernel_spmd(prog.nc, in_maps, list(range(N_CORES)),
                               trace=trace)
    z = np.concatenate([res.results[c]["z"] for c in range(N_CORES)], axis=0)
    return z.astype(np.float32), res


def kernel(**inputs):
    z, _ = _run(inputs, trace=False)
    return z


# ---------------------------------------------------------------------------
# host-side mirror of the schedule (debug aid, not used by the harness)
# ---------------------------------------------------------------------------


def _mirror(inputs, nb=32):
    """Simulate the scheduled circuit from the folded coefficient planes in
    float64 numpy; returns z [nb,12] float32."""
    gco, cco = _host_coeffs(np.asarray(inputs["x"])[:nb],
                            inputs["q_params_rot"], inputs["q_params_enta"])

    def gmat(co):
        # co [...,12] -> complex a,b,c,d from planes 0,1 / 3,4 / 6,7 / 9,10
        a = co[..., 0] + 1j * co[..., 1]
        b = co[..., 3] + 1j * co[..., 4]
        c = co[..., 6] + 1j * co[..., 7]
        d = co[..., 9] + 1j * co[..., 10]
        return np.stack([np.stack([a, b], -1), np.stack([c, d], -1)], -2)

    state = np.zeros((nb, DIM), np.complex128)
    state[:, 0] = 1.0

    def ap_1q(U, q):  # U [B,2,2]
        nonlocal state
        s = 1 << (N_QUBITS - 1 - q)
        v = state.reshape(nb, -1, 2, s)
        state = np.einsum('bij,bljr->blir', U, v).reshape(nb, DIM)

    def ap_ctrl(P0, P1, c, t):  # P0/P1 [B,2,2] or [2,2]; P0 None = identity
        nonlocal state
        v = state.reshape((nb,) + (2,) * N_QUBITS)
        v = np.moveaxis(v, (c + 1, t + 1), (1, 2)).reshape(nb, 2, 2, -1)
        def app(P, sv):
            if P is None:
                return sv
            if P.ndim == 2:
                return np.einsum('ij,bjr->bir', P, sv)
            return np.einsum('bij,bjr->bir', P, sv)
        v = np.stack([app(P0, v[:, 0]), app(P1, v[:, 1])], 1)
        v = v.reshape((nb, 2, 2) + (2,) * (N_QUBITS - 2))
        v = np.moveaxis(v, (1, 2), (c + 1, t + 1))
        state = v.reshape(nb, DIM)

    for q in range(12):
        ap_1q(gmat(gco[q].T), q)
    for op in OPS_SCHED:
        if op[0] == "crot":
            _, c, t, cc = op
            ap_ctrl(None, gmat(cco[cc]), c, t)
        elif op[0] == "full":
            _, q, g = op
            ap_1q(gmat(gco[g].T), q)
        else:
            _, c, t, g0, g1 = op
            ap_ctrl(gmat(gco[g0].T), gmat(gco[g1].T), c, t)

    probs = np.abs(state) ** 2
    z = np.empty((nb, N_QUBITS), np.float64)
    p = probs.reshape((nb,) + (2,) * N_QUBITS)
    for i in range(N_QUBITS):
        axes = tuple(a for a in range(1, N_QUBITS + 1) if a != i + 1)
        m = p.sum(axes)
        z[:, i] = m[:, 0] - m[:, 1]
    return z.astype(np.float32)



# revision 17
# speedup vs baseline: 1.0389x; 1.0389x over previous
"""Trainium2 Bass kernel for a 12-qubit batched PennyLane-style circuit.

Circuit (per batch sample), 4 layers:
  - data-encoding RY,RX,RZ,RY per wire (per-sample angles) followed by a
    fixed Rot per wire  -> folded on host into ONE 2x2 SU(2) gate G[l,q,b]
  - CRot entangling ring CRot(q, q+1 mod 12), fixed per layer.
Then <Z_i> for each of the 12 wires.

Schedule optimization: within each layer l>=1, the per-wire gate G[l,t]
(t=1..11) is delayed and merged into the following CRot(t-1, t) as a
"uniformly-controlled pair": ctrl=0 half applies G, ctrl=1 half applies
U@G.  This rewrites the full state once instead of (full + half) twice,
cutting total gate MACs ~27%.  Layer-1's 12 gates acting on |0..0> become
a direct Kronecker build of the product state.

Distribution: pure data parallel over the batch. 4096 samples -> 8 cores
x 512 samples; each core holds its 512x4096 complex statevector in SBUF
as fp32 re/im planes, batch on partitions (4 tiles of 128 samples).

Engine split per site application (knob-tuned): TensorE applies gates as
diag-matmul accumulation into PSUM (4 matmuls of 512 per output chunk)
with ScalarE wide evictions; VectorE applies them as
activation-start + scalar_tensor_tensor chains writing a ping-pong
destination buffer directly (no copy-backs).  Plain CRots run in-place
on TensorE (eviction is the write-back; untouched ctrl=0 half stays).
"""

import numpy as np

import concourse.bass as bass
import concourse.bacc as bacc
import concourse.mybir as mybir
from concourse.tile import TileContext
from concourse.bass_utils import run_bass_kernel_spmd

F32 = mybir.dt.float32
F32R = mybir.dt.float32r
F16 = mybir.dt.float16
ALU = mybir.AluOpType

N_QUBITS = 12
DIM = 4096            # 2**12
B_FULL = 4096
N_CORES = 8
B_CORE = B_FULL // N_CORES   # 512
NBT = B_CORE // 128          # 4 batch tiles of 128 samples

# coefficient plane order per gate (12 per-partition scalars)
#  a=[0,0] b=[0,1] c=[1,0] d=[1,1] of the 2x2 complex gate
CO_ARE, CO_AIM, CO_MAIM, CO_BRE, CO_BIM, CO_MBIM, \
    CO_CRE, CO_CIM, CO_MCIM, CO_DRE, CO_DIM, CO_MDIM = range(12)
NCO = 12

NG = 81               # per-sample gates: 12 kron + 3 layers * (1 + 22)
GCO_W = NG * NCO * NBT
NCC = 15              # const crot gates: 12 layer-0 + 3 wrap
CCO_W = NCC * NCO

# ---------------------------------------------------------------------------
# engine plan knobs
PLAN = ("pe", "pe", "dve")         # per-sample site applications
CROT_PLAN = ("pe",)                # plain crot sites
COPY_ROT = ("act", "dve")          # identity copies for native plain crots
DIAG_ROT = ("dve",)                # diag builds
EVICT_ROT = ("act",)               # psum evictions
SQ_ROT = ("act",)                  # observable squares
KRON_PROD_ROT = ("act", "act", "dve")  # kron product terms
OBS_GPS_BT = (3,)                  # batch tiles whose obs folds run on GpSimd

# TERMS[plane] = ordered (ci, src_idx); src order (s0re, s0im, s1re, s1im);
# planes ordered (y0re, y0im, y1re, y1im)
TERMS = (
    ((CO_ARE, 0), (CO_MAIM, 1), (CO_BRE, 2), (CO_MBIM, 3)),
    ((CO_ARE, 1), (CO_AIM, 0), (CO_BRE, 3), (CO_BIM, 2)),
    ((CO_CRE, 0), (CO_MCIM, 1), (CO_DRE, 2), (CO_MDIM, 3)),
    ((CO_CRE, 1), (CO_CIM, 0), (CO_DRE, 3), (CO_DIM, 2)),
)
# emission order grouping matmuls by diag (amortize LDWEIGHTS) with all s0
# reads in the first half (lets in-place evictions of the s0-destined planes
# start mid-pass): (ci, plane, src)
PE_ORDER = (
    (CO_ARE, 0, 0), (CO_ARE, 1, 1), (CO_MAIM, 0, 1), (CO_AIM, 1, 0),
    (CO_CRE, 2, 0), (CO_CRE, 3, 1), (CO_MCIM, 2, 1), (CO_CIM, 3, 0),
    (CO_BRE, 0, 2), (CO_BRE, 1, 3), (CO_MBIM, 0, 3), (CO_BIM, 1, 2),
    (CO_DRE, 2, 2), (CO_DRE, 3, 3), (CO_MDIM, 2, 3), (CO_DIM, 3, 2),
)


def _sched():
    """Gate schedule shared by host coeff packing and device emission.
    ('crot', c, t, cidx) | ('full', q, g) | ('pair', c, t, g0, g1).
    Each layer's G[l,t] (t>=1) merges into crot(t-1,t) of layer l; the wrap
    sites (stride-2 access) stay plain crots and G[l,0] stays a full site."""
    ops = []
    cc = 0
    for c in range(12):
        ops.append(("crot", c, (c + 1) % 12, cc))
        cc += 1
    g = 12
    for _l in (1, 2, 3):
        ops.append(("full", 0, g))
        g += 1
        for c in range(11):
            ops.append(("pair", c, c + 1, g, g + 1))
            g += 2
        ops.append(("crot", 11, 0, cc))
        cc += 1
    assert g == NG and cc == NCC
    return tuple(ops)


OPS_SCHED = _sched()

# ---------------------------------------------------------------------------
# host-side gate algebra (numpy, trivially cheap vs the device work)
# ---------------------------------------------------------------------------


def _rz(t):
    e = np.exp(-0.5j * t)
    z = np.zeros_like(e)
    return np.stack([np.stack([e, z], -1), np.stack([z, np.conj(e)], -1)], -2)


def _ry(t):
    c = np.cos(t / 2).astype(np.complex128)
    s = np.sin(t / 2).astype(np.complex128)
    return np.stack([np.stack([c, -s], -1), np.stack([s, c], -1)], -2)


def _rx(t):
    c = np.cos(t / 2).astype(np.complex128)
    s = np.sin(t / 2).astype(np.complex128)
    return np.stack([np.stack([c, -1j * s], -1), np.stack([-1j * s, c], -1)], -2)


def _rot(phi, theta, omega):
    # PennyLane Rot = RZ(omega) @ RY(theta) @ RZ(phi)
    return _rz(omega) @ _ry(theta) @ _rz(phi)


def _coef_planes(g):
    """g: [..., 2, 2] complex -> [..., 12] float32 coefficient planes."""
    a, b = g[..., 0, 0], g[..., 0, 1]
    c, d = g[..., 1, 0], g[..., 1, 1]
    cols = [a.real, a.imag, -a.imag, b.real, b.imag, -b.imag,
            c.real, c.imag, -c.imag, d.real, d.imag, -d.imag]
    return np.stack(cols, -1).astype(np.float32)


def _host_coeffs(x, q_params_rot, q_params_enta):
    """Returns (gco [NG,12,B] f32 per-sample planes, cco [NCC,12] f32)."""
    x = np.asarray(x, np.float64)
    pr = np.asarray(q_params_rot, np.float64)
    pe = np.asarray(q_params_enta, np.float64)
    B = x.shape[0]

    # per-sample encoding gate per wire: RY(x3) RZ(x2) RX(x1) RY(x0)
    enc = np.einsum('qbij,qbjk->qbik',
                    _ry(x[:, 3, :].T),
                    np.einsum('qbij,qbjk->qbik', _rz(x[:, 2, :].T),
                              np.einsum('qbij,qbjk->qbik',
                                        _rx(x[:, 1, :].T), _ry(x[:, 0, :].T))))
    rot = _rot(pr[..., 0], pr[..., 1], pr[..., 2])      # [L,Q,2,2]
    G = np.einsum('lqij,qbjk->lqbik', rot, enc)         # [L,Q,B,2,2]
    U = _rot(pe[..., 0], pe[..., 1], pe[..., 2])        # [L,Q,2,2]

    gates = np.empty((NG, B, 2, 2), np.complex128)
    gates[0:12] = G[0]
    g = 12
    for l in (1, 2, 3):
        gates[g] = G[l, 0]
        g += 1
        for c in range(11):
            t = c + 1
            gates[g] = G[l, t]                       # P0 (ctrl=0 half)
            gates[g + 1] = np.einsum('ij,bjk->bik', U[l, c], G[l, t])  # P1
            g += 2
    cgates = np.empty((NCC, 2, 2), np.complex128)
    cgates[0:12] = U[0]
    for i, l in enumerate((1, 2, 3)):
        cgates[12 + i] = U[l, 11]

    gco = np.moveaxis(_coef_planes(gates), -1, 1)       # [NG,12,B]
    cco = _coef_planes(cgates)                          # [NCC,12]
    return gco.astype(np.float32), cco.astype(np.float32)


# ---------------------------------------------------------------------------
# bass program
# ---------------------------------------------------------------------------


class _Prog:
    def __init__(self):
        nc = bacc.Bacc("TRN2", target_bir_lowering=False, debug=False)
        self.nc = nc
        self.gco_d = nc.declare_dram_parameter("gcoef", [128, GCO_W], F32,
                                               isOutput=False)
        self.cco_d = nc.declare_dram_parameter("ccoef", [128, CCO_W], F32,
                                               isOutput=False)
        self.idn_d = nc.declare_dram_parameter("ident", [128, 128], F16,
                                               isOutput=False)
        self.z_d = nc.declare_dram_parameter("z", [B_CORE, N_QUBITS], F32,
                                             isOutput=True)
        self._pctr = 0
        self._crotctr = 0
        self._cpctr = 0
        self._dctr = 0
        self._ectr = 0
        self._sqctr = 0
        self._kpctr = 0
        with TileContext(nc) as tc:
            self.tc = tc
            with tc.tile_pool(name="main", bufs=1) as pool, \
                    tc.tile_pool(name="dpool", bufs=144) as dpool, \
                    tc.tile_pool(name="psum", bufs=4, space="PSUM") as ppool:
                self.dpool = dpool
                self.ppool = ppool
                # state: bt-major, then comp (0=re 1=im), then 4096 amplitudes
                self.ST = pool.tile([128, NBT * 2 * DIM], F16, tag="state")
                self.SP = [pool.tile([128, 2 * DIM], F16, name=f"sp{i}",
                                     tag=f"sp{i}") for i in range(NBT)]
                self.GC = pool.tile([128, GCO_W], F32, tag="gc")
                self.CC = pool.tile([128, CCO_W], F32, tag="cc")
                self.I128 = pool.tile([128, 128], F16, tag="ident")
                self.ZT = [pool.tile([128, 16], F32, name=f"z{bt}",
                                     tag=f"z{bt}") for bt in range(NBT)]
                self.OT = pool.tile([128, 2 * 2048], F32, tag="obst")
                self.cur = [(self.ST, bt * 2 * DIM) for bt in range(NBT)]
                self.spare = [(self.SP[i], 0) for i in range(NBT)]

                nc.sync.dma_start(out=self.GC[:], in_=self.gco_d[:])
                nc.sync.dma_start(out=self.CC[:], in_=self.cco_d[:])
                nc.sync.dma_start(out=self.I128[:], in_=self.idn_d[:])

                self._emit_circuit()

                for bt in range(NBT):
                    nc.sync.dma_start(
                        out=self.z_d[bt * 128:(bt + 1) * 128, :],
                        in_=self.ZT[bt][:, 0:N_QUBITS])
        nc.compile()

    # ---- AP helpers -----------------------------------------------------

    def plane(self, buf, comp):
        """[128, 4096] AP of one re/im plane of a (tile, offset) buffer."""
        t, off = buf
        o = off + comp * DIM
        return t[:, o:o + DIM]

    def half_slices(self, buf, q):
        """(s0re, s0im, s1re, s1im) pair slices for a 1q gate on wire q."""
        s = 1 << (11 - q)
        out = []
        for bit in (0, 1):
            for comp in (0, 1):
                p = self.plane(buf, comp).rearrange(
                    "p (a c r) -> p a c r", c=2, r=s)
                out.append(p[:, :, bit, :])
        return (out[0], out[1], out[2], out[3])

    def pair_slices(self, buf, c, t, cbit):
        """(s0re, s0im, s1re, s1im): ctrl bit c == cbit, pair over target t."""
        out = []
        if c < t:      # adjacent, c = t-1
            st = 1 << (11 - t)
            for tbit in (0, 1):
                for comp in (0, 1):
                    p = self.plane(buf, comp).rearrange(
                        "p (a cc tt r) -> p a cc tt r", cc=2, tt=2, r=st)
                    out.append(p[:, :, cbit, tbit, :])
        else:          # wrap: c=11 (LSB), t=0 (MSB)
            for tbit in (0, 1):
                for comp in (0, 1):
                    p = self.plane(buf, comp).rearrange(
                        "p (tt a cc) -> p tt a cc", tt=2, cc=2)
                    out.append(p[:, tbit, :, cbit])
        return (out[0], out[1], out[2], out[3])

    def gco(self, bt, g, ci):
        idx = (g * NCO + ci) * NBT + bt
        return self.GC[:, idx:idx + 1]

    def cco(self, cc, ci):
        idx = cc * NCO + ci
        return self.CC[:, idx:idx + 1]

    @staticmethod
    def _chunk(view, idx, csz):
        """csz-wide column chunk of a slice-AP shaped [128, w] or [128,n,s]."""
        shp = view.shape[1:]
        if len(shp) == 1:
            return view[:, idx * csz:(idx + 1) * csz]
        n, s = shp
        if s >= csz:
            m = s // csz
            return view[:, idx // m, (idx % m) * csz:(idx % m + 1) * csz]
        na = csz // s
        return view[:, idx * na:(idx + 1) * na, :]

    @staticmethod
    def _match(ps_ap, dst):
        """View of a flat [128, w] psum AP matching dst's chunk geometry."""
        shp = dst.shape[1:]
        if len(shp) == 1:
            return ps_ap
        return ps_ap.rearrange("p (a r) -> p a r", r=shp[1])

    # ---- gate emission --------------------------------------------------

    def _build_diags(self, co):
        nc = self.nc
        AF = mybir.ActivationFunctionType
        D = {}
        for ci in range(NCO):
            d = self.dpool.tile([128, 128], F16, name="dg", tag="dg")
            eng = DIAG_ROT[self._dctr % len(DIAG_ROT)]
            self._dctr += 1
            if eng == "act":
                nc.scalar.activation(d[:], self.I128[:], AF.Copy, scale=co(ci))
            else:
                nc.vector.tensor_scalar(d[:], self.I128[:], co(ci),
                                        None, ALU.mult)
            D[ci] = d
        return D

    def _evict(self, ps_tile, dst):
        nc = self.nc
        ev = EVICT_ROT[self._ectr % len(EVICT_ROT)]
        self._ectr += 1
        src = self._match(ps_tile[:], dst)
        if ev == "act":
            nc.scalar.copy(dst, src)
        elif ev == "gps":
            nc.gpsimd.tensor_copy(out=dst, in_=src)
        else:
            nc.vector.tensor_copy(out=dst, in_=src)

    def _apply(self, s, d, co, width, plan, D=None, inplace=False):
        """Apply one 2x2 complex gate: reads slices s, writes slices d."""
        nc = self.nc
        AF = mybir.ActivationFunctionType
        if plan == "pe":
            if D is None:
                D = self._build_diags(co)
            ncp = width // 1024
            for cp in range(ncp):
                if ncp > 1:
                    ss = [self._chunk(v, cp, 1024) for v in s]
                    dd = [self._chunk(v, cp, 1024) for v in d]
                else:
                    ss, dd = s, d
                ps = [self.ppool.tile([128, 1024], F32, name="pp", tag="pp")
                      for _ in range(4)]
                if inplace:
                    # s0-reads first so s0-destined evictions can overlap the
                    # tail matmuls; evict only at end (dst aliases src)
                    kc = {}
                    for (ci, pl, si) in PE_ORDER:
                        for ch in range(2):
                            k = kc.get((pl, ch), 0)
                            nc.tensor.matmul(
                                out=ps[pl][:, ch * 512:(ch + 1) * 512],
                                lhsT=D[ci][:],
                                rhs=self._chunk(ss[si], ch, 512),
                                start=(k == 0), stop=(k == 3))
                            kc[(pl, ch)] = k + 1
                    for pl in range(4):
                        self._evict(ps[pl], dd[pl])
                else:
                    # plane-major; evict each plane as soon as it completes
                    # so PSUM slots free early and the pipe stays full
                    for pl in range(4):
                        for k, (ci, si) in enumerate(TERMS[pl]):
                            for ch in range(2):
                                nc.tensor.matmul(
                                    out=ps[pl][:, ch * 512:(ch + 1) * 512],
                                    lhsT=D[ci][:],
                                    rhs=self._chunk(ss[si], ch, 512),
                                    start=(k == 0), stop=(k == 3))
                        self._evict(ps[pl], dd[pl])
        else:
            # starts first (Sc), then stt rounds interleaved across planes so
            # the DVE queue never head-of-line blocks on one serial chain
            for pl in range(4):
                ci0, si0 = TERMS[pl][0]
                nc.scalar.activation(d[pl], s[si0], AF.Copy, scale=co(ci0))
            for k in (1, 2, 3):
                for pl in range(4):
                    ci, si = TERMS[pl][k]
                    nc.vector.scalar_tensor_tensor(
                        d[pl], s[si], co(ci), d[pl], ALU.mult, ALU.add)

    def _crot_native(self, bt, c, t, co):
        """Native plain crot: identity-copy ctrl=0 half, chains on ctrl=1."""
        nc = self.nc
        src, dst = self.cur[bt], self.spare[bt]
        s0 = self.pair_slices(src, c, t, 0)
        d0 = self.pair_slices(dst, c, t, 0)
        for k in range(4):
            eng = COPY_ROT[self._cpctr % len(COPY_ROT)]
            self._cpctr += 1
            if eng == "act":
                nc.scalar.copy(d0[k], s0[k])
            elif eng == "gps":
                nc.gpsimd.tensor_copy(out=d0[k], in_=s0[k])
            else:
                nc.vector.tensor_copy(out=d0[k], in_=s0[k])
        s1 = self.pair_slices(src, c, t, 1)
        d1 = self.pair_slices(dst, c, t, 1)
        self._apply(s1, d1, co, DIM // 4, "dve")
        self.cur[bt], self.spare[bt] = self.spare[bt], self.cur[bt]

    def _kron_init_all(self):
        """Build the layer-0 post-1q product states directly:
        state = kron_q (G[0,q] @ e0).  Step-major across batch tiles so the
        four serial doubling chains overlap."""
        nc = self.nc
        AF = mybir.ActivationFunctionType
        eng = nc.vector

        def prod(dst, src, sc):
            pe = KRON_PROD_ROT[self._kpctr % len(KRON_PROD_ROT)]
            self._kpctr += 1
            if pe == "act":
                nc.scalar.activation(dst, src, AF.Copy, scale=sc)
            else:
                eng.tensor_scalar(dst, src, sc, None, ALU.mult)

        for bt in range(NBT):
            re = self.plane(self.cur[bt], 0)
            im = self.plane(self.cur[bt], 1)
            co = lambda q, ci: self.gco(bt, q, ci)
            sp_t, sp_off = self.spare[bt]
            base = sp_off
            t0 = sp_t[:, base:base + 512]
            t1 = sp_t[:, base + 512:base + 1024]
            eng.tensor_copy(out=re[:, 0:1], in_=co(11, CO_ARE))
            eng.tensor_copy(out=im[:, 0:1], in_=co(11, CO_AIM))
            eng.tensor_copy(out=re[:, 1:2], in_=co(11, CO_CRE))
            eng.tensor_copy(out=im[:, 1:2], in_=co(11, CO_CIM))
            w = 2
            for q in range(10, -1, -1):
                csz = min(w, 512)
                for k in range(w // csz):
                    sl = slice(k * csz, (k + 1) * csz)
                    su = slice(w + k * csz, w + (k + 1) * csz)
                    ore, oim = re[:, sl], im[:, sl]
                    tt0, tt1 = t0[:, 0:csz], t1[:, 0:csz]
                    # upper half <- (c) * old (written before old clobbered)
                    prod(tt0, ore, co(q, CO_CRE))
                    eng.scalar_tensor_tensor(re[:, su], oim, co(q, CO_MCIM),
                                             tt0, ALU.mult, ALU.add)
                    prod(tt1, ore, co(q, CO_CIM))
                    eng.scalar_tensor_tensor(im[:, su], oim, co(q, CO_CRE),
                                             tt1, ALU.mult, ALU.add)
                    # lower half <- (a) * old, in place
                    prod(tt0, ore, co(q, CO_ARE))
                    prod(tt1, ore, co(q, CO_AIM))
                    eng.scalar_tensor_tensor(ore, oim, co(q, CO_MAIM),
                                             tt0, ALU.mult, ALU.add)
                    eng.scalar_tensor_tensor(oim, oim, co(q, CO_ARE),
                                             tt1, ALU.mult, ALU.add)
                w *= 2

    def _obs_ctx(self, bt):
        re = self.plane(self.cur[bt], 0)
        im = self.plane(self.cur[bt], 1)
        base = (bt % 2) * 2048
        return re, im, self.OT[:, base:base + 1024], self.OT[:, base + 1024:base + 2048]

    def _observables(self, bt):
        """probs = re^2+im^2 (overwrites re), then the 12 <Z_q> per wire."""
        nc = self.nc
        eng = nc.vector
        e = nc.gpsimd if bt in OBS_GPS_BT else eng
        AF = mybir.ActivationFunctionType
        re, im, t0, t1 = self._obs_ctx(bt)
        for h in range(4):
            sl = slice(h * 1024, (h + 1) * 1024)
            nc.scalar.activation(t0, re[:, sl], AF.Square)
            nc.scalar.activation(t1, im[:, sl], AF.Square)
            e.tensor_tensor(re[:, sl], t0, t1, ALU.add)
        # fold out qubits MSB-first; z_q = sum(lo half) - sum(hi half)
        w = DIM
        for q in range(N_QUBITS):
            h = w // 2
            lo, hi = re[:, 0:h], re[:, h:w]
            if h > 1024:  # only q=0: do the diff/reduce in two chunks
                for k in range(2):
                    sk = slice(k * 1024, (k + 1) * 1024)
                    e.tensor_tensor(t0, lo[:, sk], hi[:, sk], ALU.subtract)
                    eng.tensor_reduce(out=self.ZT[bt][:, 12 + k:13 + k],
                                      in_=t0, op=ALU.add,
                                      axis=mybir.AxisListType.X)
                eng.tensor_tensor(self.ZT[bt][:, q:q + 1],
                                  self.ZT[bt][:, 12:13],
                                  self.ZT[bt][:, 13:14], ALU.add)
            else:
                e.tensor_tensor(t0[:, 0:h], lo, hi, ALU.subtract)
                eng.tensor_reduce(out=self.ZT[bt][:, q:q + 1],
                                  in_=t0[:, 0:h], op=ALU.add,
                                  axis=mybir.AxisListType.X)
            if q < N_QUBITS - 1:
                for k in range(max(1, h // 1024)):
                    sk = slice(k * 1024, min((k + 1) * 1024, h))
                    e.tensor_tensor(lo[:, sk], lo[:, sk], hi[:, sk], ALU.add)
            w = h

    def _emit_circuit(self):
        # pre-warm: diag builds are state-independent; emit the first pe-crot
        # sites' builds before kron so the PE can start the moment kron(bt0)
        # lands
        crot_h = {}
        ci = 0
        for op in OPS_SCHED:
            if op[0] != "crot":
                continue
            plan = CROT_PLAN[ci % len(CROT_PLAN)]
            if plan == "pe" and len(crot_h) < 4:
                cc = op[3]
                co = lambda x, cc=cc: self.cco(cc, x)
                crot_h[op] = {"D": self._build_diags(co)}
            ci += 1
        self._kron_init_all()
        # app pipeline: emit each app's diag builds (state-independent) one
        # app ahead of its gate ops, hiding build latency behind prior work
        prevq = []

        def push(build_fn, apply_fn):
            D = build_fn() if build_fn else None
            if len(prevq) >= 3:
                prevq.pop(0)()
            prevq.append(lambda f=apply_fn, D=D: f(D))

        for oi, op in enumerate(OPS_SCHED):
            tail = oi >= len(OPS_SCHED) - 1
            if op[0] == "crot":
                _, c, t, cc = op
                plan = CROT_PLAN[self._crotctr % len(CROT_PLAN)]
                self._crotctr += 1
                if tail:
                    plan = "pe"
                co = lambda ci, cc=cc: self.cco(cc, ci)
                if plan == "pe":
                    h = crot_h.get(op, {})
                    for bt in range(NBT):
                        bf = None
                        if bt == 0:
                            bf = (lambda co=co, h=h:
                                  h.setdefault("D", self._build_diags(co)))

                        def ap(D, bt=bt, c=c, t=t, co=co, h=h):
                            s = self.pair_slices(self.cur[bt], c, t, 1)
                            self._apply(s, s, co, DIM // 4, "pe",
                                        D=h["D"], inplace=True)
                        push(bf, ap)
                        if tail:
                            push(None, lambda D, bt=bt: self._observables(bt))
                else:
                    for bt in range(NBT):
                        push(None, lambda D, bt=bt, c=c, t=t, co=co:
                             self._crot_native(bt, c, t, co))
                        if tail:
                            push(None, lambda D, bt=bt: self._observables(bt))
            elif op[0] == "full":
                _, q, g = op
                for bt in range(NBT):
                    plan = PLAN[self._pctr % len(PLAN)]
                    self._pctr += 1
                    co = lambda ci, bt=bt, g=g: self.gco(bt, g, ci)
                    bf = ((lambda co=co: self._build_diags(co))
                          if plan == "pe" else None)

                    def ap(D, bt=bt, q=q, co=co, plan=plan):
                        src, dst = self.cur[bt], self.spare[bt]
                        s = self.half_slices(src, q)
                        d = self.half_slices(dst, q)
                        self._apply(s, d, co, DIM // 2, plan, D=D)
                        self.cur[bt], self.spare[bt] = self.spare[bt], self.cur[bt]
                    push(bf, ap)
            else:
                _, c, t, g0, g1 = op
                for bt in range(NBT):
                    plan = PLAN[self._pctr % len(PLAN)]
                    self._pctr += 1
                    if tail:
                        plan = "pe"
                    co0 = lambda ci, bt=bt, g=g0: self.gco(bt, g, ci)
                    co1 = lambda ci, bt=bt, g=g1: self.gco(bt, g, ci)
                    bf = ((lambda co0=co0, co1=co1:
                           (self._build_diags(co0), self._build_diags(co1)))
                          if plan == "pe" else None)

                    def ap(D, bt=bt, c=c, t=t, co0=co0, co1=co1, plan=plan):
                        src, dst = self.cur[bt], self.spare[bt]
                        D0, D1 = D if D is not None else (None, None)
                        for cbit, co_, DD in ((0, co0, D0), (1, co1, D1)):
                            s = self.pair_slices(src, c, t, cbit)
                            d = self.pair_slices(dst, c, t, cbit)
                            self._apply(s, d, co_, DIM // 4, plan, D=DD)
                        self.cur[bt], self.spare[bt] = self.spare[bt], self.cur[bt]
                    push(bf, ap)
        for f in prevq:
            f()


_PROG_CACHE = None


def _get_prog():
    global _PROG_CACHE
    if _PROG_CACHE is None:
        _PROG_CACHE = _Prog()
    return _PROG_CACHE


def _run(inputs, trace=False):
    x = np.asarray(inputs["x"], np.float32)
    gco, cco = _host_coeffs(x, inputs["q_params_rot"], inputs["q_params_enta"])
    in_maps = []
    cco_tile = np.broadcast_to(
        cco.reshape(1, CCO_W), (128, CCO_W)).copy()
    for core in range(N_CORES):
        lo = core * B_CORE
        g = gco[:, :, lo:lo + B_CORE]                    # [NG,12,512]
        g = g.reshape(NG, NCO, NBT, 128)                 # [NG,12,bt,p]
        g = np.ascontiguousarray(np.moveaxis(g, -1, 0))  # [p,NG,12,bt]
        in_maps.append({
            "gcoef": g.reshape(128, GCO_W),
            "ccoef": cco_tile,
            "ident": np.eye(128, dtype=np.float16),
        })
    prog = _get_prog()
    res = run_bass_kernel_spmd(prog.nc, in_maps, list(range(N_CORES)),
                               trace=trace)
    z = np.concatenate([res.results[c]["z"] for c in range(N_CORES)], axis=0)
    return z.astype(np.float32), res


def kernel(**inputs):
    z, _ = _run(inputs, trace=False)
    return z


# ---------------------------------------------------------------------------
# host-side mirror of the schedule (debug aid, not used by the harness)
# ---------------------------------------------------------------------------


def _mirror(inputs, nb=32):
    """Simulate the scheduled circuit from the folded coefficient planes in
    float64 numpy; returns z [nb,12] float32."""
    gco, cco = _host_coeffs(np.asarray(inputs["x"])[:nb],
                            inputs["q_params_rot"], inputs["q_params_enta"])

    def gmat(co):
        # co [...,12] -> complex a,b,c,d from planes 0,1 / 3,4 / 6,7 / 9,10
        a = co[..., 0] + 1j * co[..., 1]
        b = co[..., 3] + 1j * co[..., 4]
        c = co[..., 6] + 1j * co[..., 7]
        d = co[..., 9] + 1j * co[..., 10]
        return np.stack([np.stack([a, b], -1), np.stack([c, d], -1)], -2)

    state = np.zeros((nb, DIM), np.complex128)
    state[:, 0] = 1.0

    def ap_1q(U, q):  # U [B,2,2]
        nonlocal state
        s = 1 << (N_QUBITS - 1 - q)
        v = state.reshape(nb, -1, 2, s)
        state = np.einsum('bij,bljr->blir', U, v).reshape(nb, DIM)

    def ap_ctrl(P0, P1, c, t):  # P0/P1 [B,2,2] or [2,2]; P0 None = identity
        nonlocal state
        v = state.reshape((nb,) + (2,) * N_QUBITS)
        v = np.moveaxis(v, (c + 1, t + 1), (1, 2)).reshape(nb, 2, 2, -1)
        def app(P, sv):
            if P is None:
                return sv
            if P.ndim == 2:
                return np.einsum('ij,bjr->bir', P, sv)
            return np.einsum('bij,bjr->bir', P, sv)
        v = np.stack([app(P0, v[:, 0]), app(P1, v[:, 1])], 1)
        v = v.reshape((nb, 2, 2) + (2,) * (N_QUBITS - 2))
        v = np.moveaxis(v, (1, 2), (c + 1, t + 1))
        state = v.reshape(nb, DIM)

    for q in range(12):
        ap_1q(gmat(gco[q].T), q)
    for op in OPS_SCHED:
        if op[0] == "crot":
            _, c, t, cc = op
            ap_ctrl(None, gmat(cco[cc]), c, t)
        elif op[0] == "full":
            _, q, g = op
            ap_1q(gmat(gco[g].T), q)
        else:
            _, c, t, g0, g1 = op
            ap_ctrl(gmat(gco[g0].T), gmat(gco[g1].T), c, t)

    probs = np.abs(state) ** 2
    z = np.empty((nb, N_QUBITS), np.float64)
    p = probs.reshape((nb,) + (2,) * N_QUBITS)
    for i in range(N_QUBITS):
        axes = tuple(a for a in range(1, N_QUBITS + 1) if a != i + 1)
        m = p.sum(axes)
        z[:, i] = m[:, 0] - m[:, 1]
    return z.astype(np.float32)



# revision 19
# speedup vs baseline: 1.0437x; 1.0047x over previous
"""Trainium2 Bass kernel for a 12-qubit batched PennyLane-style circuit.

Circuit (per batch sample), 4 layers:
  - data-encoding RY,RX,RZ,RY per wire (per-sample angles) followed by a
    fixed Rot per wire  -> folded on host into ONE 2x2 SU(2) gate G[l,q,b]
  - CRot entangling ring CRot(q, q+1 mod 12), fixed per layer.
Then <Z_i> for each of the 12 wires.

Schedule optimization: within each layer l>=1, the per-wire gate G[l,t]
(t=1..11) is delayed and merged into the following CRot(t-1, t) as a
"uniformly-controlled pair": ctrl=0 half applies G, ctrl=1 half applies
U@G.  This rewrites the full state once instead of (full + half) twice,
cutting total gate MACs ~27%.  Layer-1's 12 gates acting on |0..0> become
a direct Kronecker build of the product state.

Distribution: pure data parallel over the batch. 4096 samples -> 8 cores
x 512 samples; each core holds its 512x4096 complex statevector in SBUF
as fp32 re/im planes, batch on partitions (4 tiles of 128 samples).

Engine split per site application (knob-tuned): TensorE applies gates as
diag-matmul accumulation into PSUM (4 matmuls of 512 per output chunk)
with ScalarE wide evictions; VectorE applies them as
activation-start + scalar_tensor_tensor chains writing a ping-pong
destination buffer directly (no copy-backs).  Plain CRots run in-place
on TensorE (eviction is the write-back; untouched ctrl=0 half stays).
"""

import numpy as np

import concourse.bass as bass
import concourse.bacc as bacc
import concourse.mybir as mybir
from concourse.tile import TileContext
from concourse.bass_utils import run_bass_kernel_spmd

F32 = mybir.dt.float32
F32R = mybir.dt.float32r
F16 = mybir.dt.float16
ALU = mybir.AluOpType

N_QUBITS = 12
DIM = 4096            # 2**12
B_FULL = 4096
N_CORES = 8
B_CORE = B_FULL // N_CORES   # 512
NBT = B_CORE // 128          # 4 batch tiles of 128 samples

# coefficient plane order per gate (12 per-partition scalars)
#  a=[0,0] b=[0,1] c=[1,0] d=[1,1] of the 2x2 complex gate
CO_ARE, CO_AIM, CO_MAIM, CO_BRE, CO_BIM, CO_MBIM, \
    CO_CRE, CO_CIM, CO_MCIM, CO_DRE, CO_DIM, CO_MDIM = range(12)
NCO = 12

NG = 81               # per-sample gates: 12 kron + 3 layers * (1 + 22)
GCO_W = NG * NCO * NBT
NCC = 15              # const crot gates: 12 layer-0 + 3 wrap
CCO_W = NCC * NCO

# ---------------------------------------------------------------------------
# engine plan knobs
PLAN = ("pe", "pe", "dve")         # per-sample site applications
CROT_PLAN = ("pe",)                # plain crot sites
COPY_ROT = ("act", "dve")          # identity copies for native plain crots
DIAG_ROT = ("dve",)                # diag builds
EVICT_ROT = ("act",)               # psum evictions
SQ_ROT = ("act",)                  # observable squares
KRON_PROD_ROT = ("act", "act", "dve")  # kron product terms
OBS_GPS_BT = (3,)                  # batch tiles whose obs folds run on GpSimd

# TERMS[plane] = ordered (ci, src_idx); src order (s0re, s0im, s1re, s1im);
# planes ordered (y0re, y0im, y1re, y1im)
TERMS = (
    ((CO_ARE, 0), (CO_MAIM, 1), (CO_BRE, 2), (CO_MBIM, 3)),
    ((CO_ARE, 1), (CO_AIM, 0), (CO_BRE, 3), (CO_BIM, 2)),
    ((CO_CRE, 0), (CO_MCIM, 1), (CO_DRE, 2), (CO_MDIM, 3)),
    ((CO_CRE, 1), (CO_CIM, 0), (CO_DRE, 3), (CO_DIM, 2)),
)
# emission order grouping matmuls by diag (amortize LDWEIGHTS) with all s0
# reads in the first half (lets in-place evictions of the s0-destined planes
# start mid-pass): (ci, plane, src)
PE_ORDER = (
    (CO_ARE, 0, 0), (CO_ARE, 1, 1), (CO_MAIM, 0, 1), (CO_AIM, 1, 0),
    (CO_CRE, 2, 0), (CO_CRE, 3, 1), (CO_MCIM, 2, 1), (CO_CIM, 3, 0),
    (CO_BRE, 0, 2), (CO_BRE, 1, 3), (CO_MBIM, 0, 3), (CO_BIM, 1, 2),
    (CO_DRE, 2, 2), (CO_DRE, 3, 3), (CO_MDIM, 2, 3), (CO_DIM, 3, 2),
)


def _sched():
    """Gate schedule shared by host coeff packing and device emission.
    ('crot', c, t, cidx) | ('full', q, g) | ('pair', c, t, g0, g1).
    Each layer's G[l,t] (t>=1) merges into crot(t-1,t) of layer l; the wrap
    sites (stride-2 access) stay plain crots and G[l,0] stays a full site."""
    ops = []
    cc = 0
    for c in range(12):
        ops.append(("crot", c, (c + 1) % 12, cc))
        cc += 1
    g = 12
    for _l in (1, 2, 3):
        ops.append(("full", 0, g))
        g += 1
        for c in range(11):
            ops.append(("pair", c, c + 1, g, g + 1))
            g += 2
        ops.append(("crot", 11, 0, cc))
        cc += 1
    assert g == NG and cc == NCC
    return tuple(ops)


OPS_SCHED = _sched()

# ---------------------------------------------------------------------------
# host-side gate algebra (numpy, trivially cheap vs the device work)
# ---------------------------------------------------------------------------


def _rz(t):
    e = np.exp(-0.5j * t)
    z = np.zeros_like(e)
    return np.stack([np.stack([e, z], -1), np.stack([z, np.conj(e)], -1)], -2)


def _ry(t):
    c = np.cos(t / 2).astype(np.complex128)
    s = np.sin(t / 2).astype(np.complex128)
    return np.stack([np.stack([c, -s], -1), np.stack([s, c], -1)], -2)


def _rx(t):
    c = np.cos(t / 2).astype(np.complex128)
    s = np.sin(t / 2).astype(np.complex128)
    return np.stack([np.stack([c, -1j * s], -1), np.stack([-1j * s, c], -1)], -2)


def _rot(phi, theta, omega):
    # PennyLane Rot = RZ(omega) @ RY(theta) @ RZ(phi)
    return _rz(omega) @ _ry(theta) @ _rz(phi)


def _coef_planes(g):
    """g: [..., 2, 2] complex -> [..., 12] float32 coefficient planes."""
    a, b = g[..., 0, 0], g[..., 0, 1]
    c, d = g[..., 1, 0], g[..., 1, 1]
    cols = [a.real, a.imag, -a.imag, b.real, b.imag, -b.imag,
            c.real, c.imag, -c.imag, d.real, d.imag, -d.imag]
    return np.stack(cols, -1).astype(np.float32)


def _host_coeffs(x, q_params_rot, q_params_enta):
    """Returns (gco [NG,12,B] f32 per-sample planes, cco [NCC,12] f32)."""
    x = np.asarray(x, np.float64)
    pr = np.asarray(q_params_rot, np.float64)
    pe = np.asarray(q_params_enta, np.float64)
    B = x.shape[0]

    # per-sample encoding gate per wire: RY(x3) RZ(x2) RX(x1) RY(x0)
    enc = np.einsum('qbij,qbjk->qbik',
                    _ry(x[:, 3, :].T),
                    np.einsum('qbij,qbjk->qbik', _rz(x[:, 2, :].T),
                              np.einsum('qbij,qbjk->qbik',
                                        _rx(x[:, 1, :].T), _ry(x[:, 0, :].T))))
    rot = _rot(pr[..., 0], pr[..., 1], pr[..., 2])      # [L,Q,2,2]
    G = np.einsum('lqij,qbjk->lqbik', rot, enc)         # [L,Q,B,2,2]
    U = _rot(pe[..., 0], pe[..., 1], pe[..., 2])        # [L,Q,2,2]

    gates = np.empty((NG, B, 2, 2), np.complex128)
    gates[0:12] = G[0]
    g = 12
    for l in (1, 2, 3):
        gates[g] = G[l, 0]
        g += 1
        for c in range(11):
            t = c + 1
            gates[g] = G[l, t]                       # P0 (ctrl=0 half)
            gates[g + 1] = np.einsum('ij,bjk->bik', U[l, c], G[l, t])  # P1
            g += 2
    cgates = np.empty((NCC, 2, 2), np.complex128)
    cgates[0:12] = U[0]
    for i, l in enumerate((1, 2, 3)):
        cgates[12 + i] = U[l, 11]

    gco = np.moveaxis(_coef_planes(gates), -1, 1)       # [NG,12,B]
    cco = _coef_planes(cgates)                          # [NCC,12]
    return gco.astype(np.float32), cco.astype(np.float32)


# ---------------------------------------------------------------------------
# bass program
# ---------------------------------------------------------------------------


class _Prog:
    def __init__(self):
        nc = bacc.Bacc("TRN2", target_bir_lowering=False, debug=False)
        self.nc = nc
        self.gco_d = nc.declare_dram_parameter("gcoef", [128, GCO_W], F32,
                                               isOutput=False)
        self.cco_d = nc.declare_dram_parameter("ccoef", [128, CCO_W], F32,
                                               isOutput=False)
        self.idn_d = nc.declare_dram_parameter("ident", [128, 128], F16,
                                               isOutput=False)
        self.z_d = nc.declare_dram_parameter("z", [B_CORE, N_QUBITS], F32,
                                             isOutput=True)
        self._pctr = 0
        self._crotctr = 0
        self._cpctr = 0
        self._dctr = 0
        self._ectr = 0
        self._sqctr = 0
        self._kpctr = 0
        with TileContext(nc) as tc:
            self.tc = tc
            with tc.tile_pool(name="main", bufs=1) as pool, \
                    tc.tile_pool(name="dpool", bufs=16) as dpool, \
                    tc.tile_pool(name="dapool", bufs=8) as dapool, \
                    tc.tile_pool(name="psum", bufs=4, space="PSUM") as ppool:
                self.dpool = dpool
                self.dapool = dapool
                self.ppool = ppool
                # state: bt-major, then comp (0=re 1=im), then 4096 amplitudes
                self.ST = pool.tile([128, NBT * 2 * DIM], F16, tag="state")
                self.SP = [pool.tile([128, 2 * DIM], F16, name=f"sp{i}",
                                     tag=f"sp{i}") for i in range(NBT)]
                self.GC = pool.tile([128, GCO_W], F32, tag="gc")
                self.CC = pool.tile([128, CCO_W], F32, tag="cc")
                self.GC16 = pool.tile([128, GCO_W], F16, tag="gc16")
                self.CC16 = pool.tile([128, CCO_W], F16, tag="cc16")
                self.I128 = pool.tile([128, 128], F16, tag="ident")
                self.ZT = [pool.tile([128, 16], F32, name=f"z{bt}",
                                     tag=f"z{bt}") for bt in range(NBT)]
                self.OT = pool.tile([128, 2 * 2048], F32, tag="obst")
                self.cur = [(self.ST, bt * 2 * DIM) for bt in range(NBT)]
                self.spare = [(self.SP[i], 0) for i in range(NBT)]

                nc.sync.dma_start(out=self.GC[:], in_=self.gco_d[:])
                nc.sync.dma_start(out=self.CC[:], in_=self.cco_d[:])
                nc.vector.tensor_copy(out=self.GC16[:], in_=self.GC[:])
                nc.vector.tensor_copy(out=self.CC16[:], in_=self.CC[:])
                nc.sync.dma_start(out=self.I128[:], in_=self.idn_d[:])

                self._emit_circuit()

                for bt in range(NBT):
                    nc.sync.dma_start(
                        out=self.z_d[bt * 128:(bt + 1) * 128, :],
                        in_=self.ZT[bt][:, 0:N_QUBITS])
        nc.compile()

    # ---- AP helpers -----------------------------------------------------

    def plane(self, buf, comp):
        """[128, 4096] AP of one re/im plane of a (tile, offset) buffer."""
        t, off = buf
        o = off + comp * DIM
        return t[:, o:o + DIM]

    def half_slices(self, buf, q):
        """(s0re, s0im, s1re, s1im) pair slices for a 1q gate on wire q."""
        s = 1 << (11 - q)
        out = []
        for bit in (0, 1):
            for comp in (0, 1):
                p = self.plane(buf, comp).rearrange(
                    "p (a c r) -> p a c r", c=2, r=s)
                out.append(p[:, :, bit, :])
        return (out[0], out[1], out[2], out[3])

    def pair_slices(self, buf, c, t, cbit):
        """(s0re, s0im, s1re, s1im): ctrl bit c == cbit, pair over target t."""
        out = []
        if c < t:      # adjacent, c = t-1
            st = 1 << (11 - t)
            for tbit in (0, 1):
                for comp in (0, 1):
                    p = self.plane(buf, comp).rearrange(
                        "p (a cc tt r) -> p a cc tt r", cc=2, tt=2, r=st)
                    out.append(p[:, :, cbit, tbit, :])
        else:          # wrap: c=11 (LSB), t=0 (MSB)
            for tbit in (0, 1):
                for comp in (0, 1):
                    p = self.plane(buf, comp).rearrange(
                        "p (tt a cc) -> p tt a cc", tt=2, cc=2)
                    out.append(p[:, tbit, :, cbit])
        return (out[0], out[1], out[2], out[3])

    def gco(self, bt, g, ci):
        idx = (g * NCO + ci) * NBT + bt
        return self.GC[:, idx:idx + 1]

    def cco(self, cc, ci):
        idx = cc * NCO + ci
        return self.CC[:, idx:idx + 1]

    @staticmethod
    def _chunk(view, idx, csz):
        """csz-wide column chunk of a slice-AP shaped [128, w] or [128,n,s]."""
        shp = view.shape[1:]
        if len(shp) == 1:
            return view[:, idx * csz:(idx + 1) * csz]
        n, s = shp
        if s >= csz:
            m = s // csz
            return view[:, idx // m, (idx % m) * csz:(idx % m + 1) * csz]
        na = csz // s
        return view[:, idx * na:(idx + 1) * na, :]

    @staticmethod
    def _match(ps_ap, dst):
        """View of a flat [128, w] psum AP matching dst's chunk geometry."""
        shp = dst.shape[1:]
        if len(shp) == 1:
            return ps_ap
        return ps_ap.rearrange("p (a r) -> p a r", r=shp[1])

    # ---- gate emission --------------------------------------------------

    def _build_diags(self, co, gkey=None):
        nc = self.nc
        AF = mybir.ActivationFunctionType
        if gkey is not None:
            # one wide DVE op builds all 12 diagonals: out[p, ci, c] =
            # I[p, c] * coef16[p, ci] via stride-0 broadcast APs
            if gkey[0] == "g":
                _, bt, g = gkey
                base = g * NCO * NBT
                co3 = self.GC16[:, base:base + NCO * NBT].rearrange(
                    "p (ci r) -> p ci r", r=NBT)[:, :, bt:bt + 1]
            else:
                _, cc = gkey
                base = cc * NCO
                co3 = self.CC16[:, base:base + NCO].rearrange(
                    "p (ci r) -> p ci r", r=1)
            da = self.dapool.tile([128, NCO * 128], F16, name="dga",
                                  tag="dga")
            i3 = self.I128[:].rearrange("p (a c) -> p a c", a=1)
            ib, cb = bass.broadcast_tensor_aps(i3, co3)
            nc.vector.tensor_tensor(
                da[:].rearrange("p (ci c) -> p ci c", c=128), ib, cb,
                ALU.mult)
            return {ci: da[:, ci * 128:(ci + 1) * 128] for ci in range(NCO)}
        D = {}
        for ci in range(NCO):
            d = self.dpool.tile([128, 128], F16, name="dg", tag="dg")
            eng = DIAG_ROT[self._dctr % len(DIAG_ROT)]
            self._dctr += 1
            if eng == "act":
                nc.scalar.activation(d[:], self.I128[:], AF.Copy, scale=co(ci))
            else:
                nc.vector.tensor_scalar(d[:], self.I128[:], co(ci),
                                        None, ALU.mult)
            D[ci] = d
        return D

    def _evict(self, ps_tile, dst):
        nc = self.nc
        ev = EVICT_ROT[self._ectr % len(EVICT_ROT)]
        self._ectr += 1
        src = self._match(ps_tile[:], dst)
        if ev == "act":
            nc.scalar.copy(dst, src)
        elif ev == "gps":
            nc.gpsimd.tensor_copy(out=dst, in_=src)
        else:
            nc.vector.tensor_copy(out=dst, in_=src)

    def _apply(self, s, d, co, width, plan, D=None, inplace=False):
        """Apply one 2x2 complex gate: reads slices s, writes slices d."""
        nc = self.nc
        AF = mybir.ActivationFunctionType
        if plan == "pe":
            if D is None:
                D = self._build_diags(co)
            ncp = width // 1024
            for cp in range(ncp):
                if ncp > 1:
                    ss = [self._chunk(v, cp, 1024) for v in s]
                    dd = [self._chunk(v, cp, 1024) for v in d]
                else:
                    ss, dd = s, d
                ps = [self.ppool.tile([128, 1024], F32, name="pp", tag="pp")
                      for _ in range(4)]
                if inplace:
                    # s0-reads first so s0-destined evictions can overlap the
                    # tail matmuls; evict only at end (dst aliases src)
                    kc = {}
                    for (ci, pl, si) in PE_ORDER:
                        for ch in range(2):
                            k = kc.get((pl, ch), 0)
                            nc.tensor.matmul(
                                out=ps[pl][:, ch * 512:(ch + 1) * 512],
                                lhsT=D[ci][:],
                                rhs=self._chunk(ss[si], ch, 512),
                                start=(k == 0), stop=(k == 3))
                            kc[(pl, ch)] = k + 1
                    for pl in range(4):
                        self._evict(ps[pl], dd[pl])
                else:
                    # plane-major; evict each plane as soon as it completes
                    # so PSUM slots free early and the pipe stays full
                    for pl in range(4):
                        for k, (ci, si) in enumerate(TERMS[pl]):
                            for ch in range(2):
                                nc.tensor.matmul(
                                    out=ps[pl][:, ch * 512:(ch + 1) * 512],
                                    lhsT=D[ci][:],
                                    rhs=self._chunk(ss[si], ch, 512),
                                    start=(k == 0), stop=(k == 3))
                        self._evict(ps[pl], dd[pl])
        else:
            # starts first (Sc), then stt rounds interleaved across planes so
            # the DVE queue never head-of-line blocks on one serial chain
            for pl in range(4):
                ci0, si0 = TERMS[pl][0]
                nc.scalar.activation(d[pl], s[si0], AF.Copy, scale=co(ci0))
            for k in (1, 2, 3):
                for pl in range(4):
                    ci, si = TERMS[pl][k]
                    nc.vector.scalar_tensor_tensor(
                        d[pl], s[si], co(ci), d[pl], ALU.mult, ALU.add)

    def _crot_native(self, bt, c, t, co):
        """Native plain crot: identity-copy ctrl=0 half, chains on ctrl=1."""
        nc = self.nc
        src, dst = self.cur[bt], self.spare[bt]
        s0 = self.pair_slices(src, c, t, 0)
        d0 = self.pair_slices(dst, c, t, 0)
        for k in range(4):
            eng = COPY_ROT[self._cpctr % len(COPY_ROT)]
            self._cpctr += 1
            if eng == "act":
                nc.scalar.copy(d0[k], s0[k])
            elif eng == "gps":
                nc.gpsimd.tensor_copy(out=d0[k], in_=s0[k])
            else:
                nc.vector.tensor_copy(out=d0[k], in_=s0[k])
        s1 = self.pair_slices(src, c, t, 1)
        d1 = self.pair_slices(dst, c, t, 1)
        self._apply(s1, d1, co, DIM // 4, "dve")
        self.cur[bt], self.spare[bt] = self.spare[bt], self.cur[bt]

    def _kron_init_all(self):
        """Build the layer-0 post-1q product states directly:
        state = kron_q (G[0,q] @ e0).  Step-major across batch tiles so the
        four serial doubling chains overlap."""
        nc = self.nc
        AF = mybir.ActivationFunctionType
        eng = nc.vector

        def prod(dst, src, sc):
            pe = KRON_PROD_ROT[self._kpctr % len(KRON_PROD_ROT)]
            self._kpctr += 1
            if pe == "act":
                nc.scalar.activation(dst, src, AF.Copy, scale=sc)
            else:
                eng.tensor_scalar(dst, src, sc, None, ALU.mult)

        for bt in range(NBT):
            re = self.plane(self.cur[bt], 0)
            im = self.plane(self.cur[bt], 1)
            co = lambda q, ci: self.gco(bt, q, ci)
            sp_t, sp_off = self.spare[bt]
            base = sp_off
            t0 = sp_t[:, base:base + 512]
            t1 = sp_t[:, base + 512:base + 1024]
            eng.tensor_copy(out=re[:, 0:1], in_=co(11, CO_ARE))
            eng.tensor_copy(out=im[:, 0:1], in_=co(11, CO_AIM))
            eng.tensor_copy(out=re[:, 1:2], in_=co(11, CO_CRE))
            eng.tensor_copy(out=im[:, 1:2], in_=co(11, CO_CIM))
            w = 2
            for q in range(10, -1, -1):
                csz = min(w, 512)
                for k in range(w // csz):
                    sl = slice(k * csz, (k + 1) * csz)
                    su = slice(w + k * csz, w + (k + 1) * csz)
                    ore, oim = re[:, sl], im[:, sl]
                    tt0, tt1 = t0[:, 0:csz], t1[:, 0:csz]
                    # upper half <- (c) * old (written before old clobbered)
                    prod(tt0, ore, co(q, CO_CRE))
                    eng.scalar_tensor_tensor(re[:, su], oim, co(q, CO_MCIM),
                                             tt0, ALU.mult, ALU.add)
                    prod(tt1, ore, co(q, CO_CIM))
                    eng.scalar_tensor_tensor(im[:, su], oim, co(q, CO_CRE),
                                             tt1, ALU.mult, ALU.add)
                    # lower half <- (a) * old, in place
                    prod(tt0, ore, co(q, CO_ARE))
                    prod(tt1, ore, co(q, CO_AIM))
                    eng.scalar_tensor_tensor(ore, oim, co(q, CO_MAIM),
                                             tt0, ALU.mult, ALU.add)
                    eng.scalar_tensor_tensor(oim, oim, co(q, CO_ARE),
                                             tt1, ALU.mult, ALU.add)
                w *= 2

    def _obs_ctx(self, bt):
        re = self.plane(self.cur[bt], 0)
        im = self.plane(self.cur[bt], 1)
        base = (bt % 2) * 2048
        return re, im, self.OT[:, base:base + 1024], self.OT[:, base + 1024:base + 2048]

    def _observables(self, bt):
        """probs = re^2+im^2 (overwrites re), then the 12 <Z_q> per wire."""
        nc = self.nc
        eng = nc.vector
        e = nc.gpsimd if bt in OBS_GPS_BT else eng
        AF = mybir.ActivationFunctionType
        re, im, t0, t1 = self._obs_ctx(bt)
        for h in range(4):
            sl = slice(h * 1024, (h + 1) * 1024)
            nc.scalar.activation(t0, re[:, sl], AF.Square)
            nc.scalar.activation(t1, im[:, sl], AF.Square)
            e.tensor_tensor(re[:, sl], t0, t1, ALU.add)
        # fold out qubits MSB-first; z_q = sum(lo half) - sum(hi half)
        w = DIM
        for q in range(N_QUBITS):
            h = w // 2
            lo, hi = re[:, 0:h], re[:, h:w]
            if h > 1024:  # only q=0: do the diff/reduce in two chunks
                for k in range(2):
                    sk = slice(k * 1024, (k + 1) * 1024)
                    e.tensor_tensor(t0, lo[:, sk], hi[:, sk], ALU.subtract)
                    eng.tensor_reduce(out=self.ZT[bt][:, 12 + k:13 + k],
                                      in_=t0, op=ALU.add,
                                      axis=mybir.AxisListType.X)
                eng.tensor_tensor(self.ZT[bt][:, q:q + 1],
                                  self.ZT[bt][:, 12:13],
                                  self.ZT[bt][:, 13:14], ALU.add)
            else:
                e.tensor_tensor(t0[:, 0:h], lo, hi, ALU.subtract)
                eng.tensor_reduce(out=self.ZT[bt][:, q:q + 1],
                                  in_=t0[:, 0:h], op=ALU.add,
                                  axis=mybir.AxisListType.X)
            if q < N_QUBITS - 1:
                for k in range(max(1, h // 1024)):
                    sk = slice(k * 1024, min((k + 1) * 1024, h))
                    e.tensor_tensor(lo[:, sk], lo[:, sk], hi[:, sk], ALU.add)
            w = h

    def _emit_circuit(self):
        # pre-warm: diag builds are state-independent; emit the first pe-crot
        # sites' builds before kron so the PE can start the moment kron(bt0)
        # lands
        crot_h = {}
        ci = 0
        for op in OPS_SCHED:
            if op[0] != "crot":
                continue
            plan = CROT_PLAN[ci % len(CROT_PLAN)]
            if plan == "pe" and len(crot_h) < 4:
                cc = op[3]
                co = lambda x, cc=cc: self.cco(cc, x)
                crot_h[op] = {"D": self._build_diags(co, gkey=("cc", cc))}
            ci += 1
        self._kron_init_all()
        # app pipeline: emit each app's diag builds (state-independent) one
        # app ahead of its gate ops, hiding build latency behind prior work
        prevq = []

        def push(build_fn, apply_fn):
            D = build_fn() if build_fn else None
            if len(prevq) >= 3:
                prevq.pop(0)()
            prevq.append(lambda f=apply_fn, D=D: f(D))

        for oi, op in enumerate(OPS_SCHED):
            tail = oi >= len(OPS_SCHED) - 1
            if op[0] == "crot":
                _, c, t, cc = op
                plan = CROT_PLAN[self._crotctr % len(CROT_PLAN)]
                self._crotctr += 1
                if tail:
                    plan = "pe"
                co = lambda ci, cc=cc: self.cco(cc, ci)
                if plan == "pe":
                    h = crot_h.get(op, {})
                    for bt in range(NBT):
                        bf = None
                        if bt == 0:
                            bf = (lambda co=co, h=h, cc=cc:
                                  h.setdefault("D", self._build_diags(
                                      co, gkey=("cc", cc))))

                        def ap(D, bt=bt, c=c, t=t, co=co, h=h):
                            s = self.pair_slices(self.cur[bt], c, t, 1)
                            self._apply(s, s, co, DIM // 4, "pe",
                                        D=h["D"], inplace=True)
                        push(bf, ap)
                        if tail:
                            push(None, lambda D, bt=bt: self._observables(bt))
                else:
                    for bt in range(NBT):
                        push(None, lambda D, bt=bt, c=c, t=t, co=co:
                             self._crot_native(bt, c, t, co))
                        if tail:
                            push(None, lambda D, bt=bt: self._observables(bt))
            elif op[0] == "full":
                _, q, g = op
                for bt in range(NBT):
                    plan = PLAN[self._pctr % len(PLAN)]
                    self._pctr += 1
                    co = lambda ci, bt=bt, g=g: self.gco(bt, g, ci)
                    bf = ((lambda co=co, bt=bt, g=g: self._build_diags(
                        co, gkey=("g", bt, g)))
                          if plan == "pe" else None)

                    def ap(D, bt=bt, q=q, co=co, plan=plan):
                        src, dst = self.cur[bt], self.spare[bt]
                        s = self.half_slices(src, q)
                        d = self.half_slices(dst, q)
                        self._apply(s, d, co, DIM // 2, plan, D=D)
                        self.cur[bt], self.spare[bt] = self.spare[bt], self.cur[bt]
                    push(bf, ap)
            else:
                _, c, t, g0, g1 = op
                for bt in range(NBT):
                    plan = PLAN[self._pctr % len(PLAN)]
                    self._pctr += 1
                    if tail:
                        plan = "pe"
                    co0 = lambda ci, bt=bt, g=g0: self.gco(bt, g, ci)
                    co1 = lambda ci, bt=bt, g=g1: self.gco(bt, g, ci)
                    bf = ((lambda co0=co0, co1=co1, bt=bt, g0=g0, g1=g1:
                           (self._build_diags(co0, gkey=("g", bt, g0)),
                            self._build_diags(co1, gkey=("g", bt, g1))))
                          if plan == "pe" else None)

                    def ap(D, bt=bt, c=c, t=t, co0=co0, co1=co1, plan=plan):
                        src, dst = self.cur[bt], self.spare[bt]
                        D0, D1 = D if D is not None else (None, None)
                        for cbit, co_, DD in ((0, co0, D0), (1, co1, D1)):
                            s = self.pair_slices(src, c, t, cbit)
                            d = self.pair_slices(dst, c, t, cbit)
                            self._apply(s, d, co_, DIM // 4, plan, D=DD)
                        self.cur[bt], self.spare[bt] = self.spare[bt], self.cur[bt]
                    push(bf, ap)
        for f in prevq:
            f()


_PROG_CACHE = None


def _get_prog():
    global _PROG_CACHE
    if _PROG_CACHE is None:
        _PROG_CACHE = _Prog()
    return _PROG_CACHE


def _run(inputs, trace=False):
    x = np.asarray(inputs["x"], np.float32)
    gco, cco = _host_coeffs(x, inputs["q_params_rot"], inputs["q_params_enta"])
    in_maps = []
    cco_tile = np.broadcast_to(
        cco.reshape(1, CCO_W), (128, CCO_W)).copy()
    for core in range(N_CORES):
        lo = core * B_CORE
        g = gco[:, :, lo:lo + B_CORE]                    # [NG,12,512]
        g = g.reshape(NG, NCO, NBT, 128)                 # [NG,12,bt,p]
        g = np.ascontiguousarray(np.moveaxis(g, -1, 0))  # [p,NG,12,bt]
        in_maps.append({
            "gcoef": g.reshape(128, GCO_W),
            "ccoef": cco_tile,
            "ident": np.eye(128, dtype=np.float16),
        })
    prog = _get_prog()
    res = run_bass_kernel_spmd(prog.nc, in_maps, list(range(N_CORES)),
                               trace=trace)
    z = np.concatenate([res.results[c]["z"] for c in range(N_CORES)], axis=0)
    return z.astype(np.float32), res


def kernel(**inputs):
    z, _ = _run(inputs, trace=False)
    return z


# ---------------------------------------------------------------------------
# host-side mirror of the schedule (debug aid, not used by the harness)
# ---------------------------------------------------------------------------


def _mirror(inputs, nb=32):
    """Simulate the scheduled circuit from the folded coefficient planes in
    float64 numpy; returns z [nb,12] float32."""
    gco, cco = _host_coeffs(np.asarray(inputs["x"])[:nb],
                            inputs["q_params_rot"], inputs["q_params_enta"])

    def gmat(co):
        # co [...,12] -> complex a,b,c,d from planes 0,1 / 3,4 / 6,7 / 9,10
        a = co[..., 0] + 1j * co[..., 1]
        b = co[..., 3] + 1j * co[..., 4]
        c = co[..., 6] + 1j * co[..., 7]
        d = co[..., 9] + 1j * co[..., 10]
        return np.stack([np.stack([a, b], -1), np.stack([c, d], -1)], -2)

    state = np.zeros((nb, DIM), np.complex128)
    state[:, 0] = 1.0

    def ap_1q(U, q):  # U [B,2,2]
        nonlocal state
        s = 1 << (N_QUBITS - 1 - q)
        v = state.reshape(nb, -1, 2, s)
        state = np.einsum('bij,bljr->blir', U, v).reshape(nb, DIM)

    def ap_ctrl(P0, P1, c, t):  # P0/P1 [B,2,2] or [2,2]; P0 None = identity
        nonlocal state
        v = state.reshape((nb,) + (2,) * N_QUBITS)
        v = np.moveaxis(v, (c + 1, t + 1), (1, 2)).reshape(nb, 2, 2, -1)
        def app(P, sv):
            if P is None:
                return sv
            if P.ndim == 2:
                return np.einsum('ij,bjr->bir', P, sv)
            return np.einsum('bij,bjr->bir', P, sv)
        v = np.stack([app(P0, v[:, 0]), app(P1, v[:, 1])], 1)
        v = v.reshape((nb, 2, 2) + (2,) * (N_QUBITS - 2))
        v = np.moveaxis(v, (1, 2), (c + 1, t + 1))
        state = v.reshape(nb, DIM)

    for q in range(12):
        ap_1q(gmat(gco[q].T), q)
    for op in OPS_SCHED:
        if op[0] == "crot":
            _, c, t, cc = op
            ap_ctrl(None, gmat(cco[cc]), c, t)
        elif op[0] == "full":
            _, q, g = op
            ap_1q(gmat(gco[g].T), q)
        else:
            _, c, t, g0, g1 = op
            ap_ctrl(gmat(gco[g0].T), gmat(gco[g1].T), c, t)

    probs = np.abs(state) ** 2
    z = np.empty((nb, N_QUBITS), np.float64)
    p = probs.reshape((nb,) + (2,) * N_QUBITS)
    for i in range(N_QUBITS):
        axes = tuple(a for a in range(1, N_QUBITS + 1) if a != i + 1)
        m = p.sum(axes)
        z[:, i] = m[:, 0] - m[:, 1]
    return z.astype(np.float32)

